# revision 1
# baseline (speedup 1.0000x reference)
import sys

sys.path.insert(0, "/opt/trn_rl_repo")

import numpy as np
import concourse.bass as bass
import concourse.bacc as bacc
import concourse.mybir as mybir
import concourse.tile as tile
from concourse.bass_utils import run_bass_kernel_spmd

F32 = mybir.dt.float32
I32 = mybir.dt.int32
I16 = mybir.dt.int16
OP = mybir.AluOpType
AF = mybir.ActivationFunctionType

B = 8
V = 8192
P = 128
NSUB = V // P
NW = V // 16

SCALES = [
    (256, 56, 56, 1.0 / 8.0),
    (512, 28, 28, 1.0 / 16.0),
    (512, 14, 14, 1.0 / 32.0),
]
COFF = [0, 256, 768]
OCH = 256
NCHUNK = V // OCH

_CACHE = {}


def _floor_pipeline(nc, sb, x, shape, tag, want_weights=True):
    ti = sb.tile(shape, I32, tag="fp_ti")
    nc.vector.tensor_copy(out=ti[:], in_=x[:])
    tf = sb.tile(shape, F32, tag="fp_tf")
    nc.vector.tensor_copy(out=tf[:], in_=ti[:])
    cmp = sb.tile(shape, F32, tag="fp_cmp")
    nc.vector.tensor_tensor(out=cmp[:], in0=tf[:], in1=x[:], op=OP.is_gt)
    fl = sb.tile(shape, F32, tag=f"{tag}_fl")
    nc.vector.tensor_tensor(out=fl[:], in0=tf[:], in1=cmp[:], op=OP.subtract)
    if not want_weights:
        return fl, None, None
    wx2 = sb.tile(shape, F32, tag=f"{tag}_wx2")
    nc.vector.tensor_tensor(out=wx2[:], in0=x[:], in1=fl[:], op=OP.subtract)
    cmp2 = sb.tile(shape, F32, tag="fp_cmp2")
    nc.vector.tensor_tensor(out=cmp2[:], in0=x[:], in1=fl[:], op=OP.is_gt)
    ce = sb.tile(shape, F32, tag="fp_ce")
    nc.vector.tensor_tensor(out=ce[:], in0=fl[:], in1=cmp2[:], op=OP.add)
    wx1 = sb.tile(shape, F32, tag=f"{tag}_wx1")
    nc.vector.tensor_tensor(out=wx1[:], in0=ce[:], in1=x[:], op=OP.subtract)
    return fl, wx2, wx1


def build():
    nc = bacc.Bacc("TRN2", target_bir_lowering=False, debug=False, num_swdge_queues=4)

    coords = nc.dram_tensor("coords", [V, 2], F32, kind="ExternalInput")
    tabs = []
    for si, (C, H, W, _) in enumerate(SCALES):
        tabs.append(
            nc.dram_tensor(f"t{si}", [(H - 1) * W, 2 * C], F32, kind="ExternalInput")
        )
    out = nc.dram_tensor("out", [V, 1280], F32, kind="ExternalOutput")

    with tile.TileContext(nc) as tc:
        with (
            tc.tile_pool(name="pre", bufs=1) as pre,
            tc.tile_pool(name="g3", bufs=2) as g3p,
            tc.tile_pool(name="g4", bufs=2) as g4p,
            tc.tile_pool(name="g5", bufs=2) as g5p,
            tc.tile_pool(name="ob", bufs=2) as obp,
            tc.tile_pool(name="tmp", bufs=4) as tmp,
        ):
            idx128 = []
            for si, (C, H, W, inv) in enumerate(SCALES):
                xw = pre.tile([16, NW], F32, tag="xw")
                yw = pre.tile([16, NW], F32, tag="yw")
                nc.sync.dma_start(out=xw[:], in_=bass.AP(coords, 0, [[2, 16], [32, NW]]))
                nc.sync.dma_start(out=yw[:], in_=bass.AP(coords, 1, [[2, 16], [32, NW]]))
                xws = pre.tile([16, NW], F32, tag="xws")
                yws = pre.tile([16, NW], F32, tag="yws")
                nc.vector.tensor_scalar(xws[:], xw[:], inv, None, OP.mult)
                nc.vector.tensor_scalar(yws[:], yw[:], inv, None, OP.mult)
                flx, _, _ = _floor_pipeline(nc, pre, xws, [16, NW], "ix", want_weights=False)
                fly, _, _ = _floor_pipeline(nc, pre, yws, [16, NW], "iy", want_weights=False)
                pidx = pre.tile([16, NW], F32, tag="pidx")
                nc.vector.tensor_scalar(pidx[:], fly[:], float(W), None, OP.mult)
                nc.vector.tensor_tensor(out=pidx[:], in0=pidx[:], in1=flx[:], op=OP.add)
                pidx16 = pre.tile([16, NW], I16, tag="pidx16")
                nc.vector.tensor_copy(out=pidx16[:], in_=pidx[:])
                full = pre.tile([128, NW], I16, tag=f"idx128_{si}")
                for g in range(8):
                    nc.sync.dma_start(out=full[16 * g : 16 * (g + 1), :], in_=pidx16[:, :])
                idx128.append(full)

            xp = pre.tile([128, NSUB], F32)
            yp = pre.tile([128, NSUB], F32)
            nc.sync.dma_start(out=xp[:], in_=bass.AP(coords, 0, [[2, 128], [256, NSUB]]))
            nc.sync.dma_start(out=yp[:], in_=bass.AP(coords, 1, [[2, 128], [256, NSUB]]))
            wts = []
            for si, (C, H, W, inv) in enumerate(SCALES):
                xs = pre.tile([128, NSUB], F32, tag="xs")
                ys = pre.tile([128, NSUB], F32, tag="ys")
                nc.vector.tensor_scalar(xs[:], xp[:], inv, None, OP.mult)
                nc.vector.tensor_scalar(ys[:], yp[:], inv, None, OP.mult)
                _, wx2, wx1 = _floor_pipeline(nc, pre, xs, [128, NSUB], "wx")
                _, wy2, wy1 = _floor_pipeline(nc, pre, ys, [128, NSUB], "wy")
                ws = []
                for (wx, wy, nm) in [
                    (wx1, wy1, "w11"),
                    (wx1, wy2, "w12"),
                    (wx2, wy1, "w21"),
                    (wx2, wy2, "w22"),
                ]:
                    w = pre.tile([128, NSUB], F32, tag=f"{nm}_{si}")
                    nc.vector.tensor_tensor(out=w[:], in0=wx[:], in1=wy[:], op=OP.mult)
                    ws.append(w)
                wts.append(ws)

            pools = [g3p, g4p, g5p]
            NS = OCH // 128
            for c in range(NCHUNK):
                slabs = []
                for si, (C, H, W, inv) in enumerate(SCALES):
                    slab = pools[si].tile([128, NS, 4 * C], F32, tag=f"slab{si}")
                    i0 = (c * OCH) // 16
                    nc.gpsimd.dma_gather(
                        out_ap=slab[:],
                        in_ap=bass.AP(tabs[si], 0, [[2 * C, (H - 1) * W - 1], [1, 4 * C]]),
                        idxs_ap=idx128[si][:, i0 : i0 + OCH // 16],
                        num_idxs=OCH,
                        num_idxs_reg=OCH,
                        elem_size=4 * C,
                        elem_step=2 * C,
                        queue_num=si,
                    )
                    slabs.append(slab)

                oslab = obp.tile([128, NS, 1280], F32, tag="oslab")
                for s in range(NS):
                    g = c * NS + s
                    for si, (C, H, W, inv) in enumerate(SCALES):
                        w11, w12, w21, w22 = wts[si]
                        slab = slabs[si]
                        t0 = tmp.tile([128, 512], F32, tag="t0")
                        t1 = tmp.tile([128, 512], F32, tag="t1")
                        t2 = tmp.tile([128, 512], F32, tag="t2")
                        t3 = tmp.tile([128, 512], F32, tag="t3")
                        nc.vector.tensor_scalar(
                            t0[:, :C], slab[:, s, 0:C], w11[:, g : g + 1], None, OP.mult
                        )
                        nc.scalar.activation(
                            t1[:, :C], slab[:, s, C : 2 * C], AF.Copy, scale=w12[:, g : g + 1]
                        )
                        nc.vector.tensor_scalar(
                            t2[:, :C], slab[:, s, 2 * C : 3 * C], w21[:, g : g + 1], None, OP.mult
                        )
                        nc.scalar.activation(
                            t3[:, :C], slab[:, s, 3 * C : 4 * C], AF.Copy, scale=w22[:, g : g + 1]
                        )
                        nc.vector.tensor_tensor(out=t0[:, :C], in0=t0[:, :C], in1=t1[:, :C], op=OP.add)
                        nc.gpsimd.tensor_tensor(out=t2[:, :C], in0=t2[:, :C], in1=t3[:, :C], op=OP.add)
                        nc.vector.tensor_tensor(
                            out=oslab[:, s, COFF[si] : COFF[si] + C],
                            in0=t0[:, :C],
                            in1=t2[:, :C],
                            op=OP.add,
                        )
                nc.sync.dma_start(
                    out=bass.AP(
                        out,
                        c * OCH * 1280,
                        [[1280, 128], [128 * 1280, NS], [1, 1280]],
                    ),
                    in_=oslab[:],
                )
    nc.compile()
    return nc


def _make_tables(fm):
    C, H, W = fm.shape
    t = np.ascontiguousarray(fm.transpose(1, 2, 0))
    rp = np.concatenate([t[:-1], t[1:]], axis=2)
    return np.ascontiguousarray(rp.reshape((H - 1) * W, 2 * C))


def kernel(c, fm3, fm4, fm5):
    c = np.asarray(c, np.float32)
    fms = [np.asarray(fm3, np.float32), np.asarray(fm4, np.float32), np.asarray(fm5, np.float32)]
    if "nc" not in _CACHE:
        _CACHE["nc"] = build()
    nc = _CACHE["nc"]
    in_maps = []
    for b in range(B):
        m = {"coords": np.ascontiguousarray(c[b])}
        for si in range(3):
            m[f"t{si}"] = _make_tables(fms[si][b])
        in_maps.append(m)
    res = run_bass_kernel_spmd(nc, in_maps, core_ids=list(range(B)))
    return np.stack([res.results[b]["out"] for b in range(B)], axis=0)



# revision 4
# speedup vs baseline: 1.8109x; 1.8109x over previous
import sys

sys.path.insert(0, "/opt/trn_rl_repo")

import numpy as np
import ml_dtypes
import concourse.bass as bass
import concourse.bacc as bacc
import concourse.mybir as mybir
import concourse.tile as tile
from concourse.bass_utils import run_bass_kernel_spmd

F32 = mybir.dt.float32
BF16 = mybir.dt.bfloat16
I32 = mybir.dt.int32
I16 = mybir.dt.int16
OP = mybir.AluOpType
AF = mybir.ActivationFunctionType

B = 8
V = 8192
P = 128
NSUB = V // P
NW = V // 16

SCALES = [
    (256, 56, 56, 1.0 / 8.0),
    (512, 28, 28, 1.0 / 16.0),
    (512, 14, 14, 1.0 / 32.0),
]
COFF = [0, 256, 768]
OCH = 512
NCHUNK = V // OCH

_CACHE = {}


def _floor_pipeline(nc, sb, x, shape, tag, want_weights=True):
    ti = sb.tile(shape, I32, tag="fp_ti")
    nc.vector.tensor_copy(out=ti[:], in_=x[:])
    tf = sb.tile(shape, F32, tag="fp_tf")
    nc.vector.tensor_copy(out=tf[:], in_=ti[:])
    cmp = sb.tile(shape, F32, tag="fp_cmp")
    nc.vector.tensor_tensor(out=cmp[:], in0=tf[:], in1=x[:], op=OP.is_gt)
    fl = sb.tile(shape, F32, tag=f"{tag}_fl")
    nc.vector.tensor_tensor(out=fl[:], in0=tf[:], in1=cmp[:], op=OP.subtract)
    if not want_weights:
        return fl, None, None
    wx2 = sb.tile(shape, F32, tag=f"{tag}_wx2")
    nc.vector.tensor_tensor(out=wx2[:], in0=x[:], in1=fl[:], op=OP.subtract)
    cmp2 = sb.tile(shape, F32, tag="fp_cmp2")
    nc.vector.tensor_tensor(out=cmp2[:], in0=x[:], in1=fl[:], op=OP.is_gt)
    ce = sb.tile(shape, F32, tag="fp_ce")
    nc.vector.tensor_tensor(out=ce[:], in0=fl[:], in1=cmp2[:], op=OP.add)
    wx1 = sb.tile(shape, F32, tag=f"{tag}_wx1")
    nc.vector.tensor_tensor(out=wx1[:], in0=ce[:], in1=x[:], op=OP.subtract)
    return fl, wx2, wx1


def build():
    nc = bacc.Bacc("TRN2", target_bir_lowering=False, debug=False, num_swdge_queues=4)

    coords = nc.dram_tensor("coords", [V, 2], F32, kind="ExternalInput")
    tabs = []
    for si, (C, H, W, _) in enumerate(SCALES):
        tabs.append(
            nc.dram_tensor(f"t{si}", [(H - 1) * W, 2 * C], BF16, kind="ExternalInput")
        )
    out = nc.dram_tensor("out", [V, 1280], BF16, kind="ExternalOutput")

    with tile.TileContext(nc) as tc:
        with (
            tc.tile_pool(name="pre", bufs=1) as pre,
            tc.tile_pool(name="g3", bufs=2) as g3p,
            tc.tile_pool(name="g4", bufs=2) as g4p,
            tc.tile_pool(name="g5", bufs=2) as g5p,
            tc.tile_pool(name="ob", bufs=2) as obp,
            tc.tile_pool(name="tmp", bufs=4) as tmp,
        ):
            idx128 = []
            for si, (C, H, W, inv) in enumerate(SCALES):
                xw = pre.tile([16, NW], F32, tag="xw")
                yw = pre.tile([16, NW], F32, tag="yw")
                nc.sync.dma_start(out=xw[:], in_=bass.AP(coords, 0, [[2, 16], [32, NW]]))
                nc.sync.dma_start(out=yw[:], in_=bass.AP(coords, 1, [[2, 16], [32, NW]]))
                xws = pre.tile([16, NW], F32, tag="xws")
                yws = pre.tile([16, NW], F32, tag="yws")
                nc.vector.tensor_scalar(xws[:], xw[:], inv, None, OP.mult)
                nc.vector.tensor_scalar(yws[:], yw[:], inv, None, OP.mult)
                flx, _, _ = _floor_pipeline(nc, pre, xws, [16, NW], "ix", want_weights=False)
                fly, _, _ = _floor_pipeline(nc, pre, yws, [16, NW], "iy", want_weights=False)
                pidx = pre.tile([16, NW], F32, tag="pidx")
                nc.vector.tensor_scalar(pidx[:], fly[:], float(W), None, OP.mult)
                nc.vector.tensor_tensor(out=pidx[:], in0=pidx[:], in1=flx[:], op=OP.add)
                pidx16 = pre.tile([16, NW], I16, tag="pidx16")
                nc.vector.tensor_copy(out=pidx16[:], in_=pidx[:])
                full = pre.tile([128, NW], I16, tag=f"idx128_{si}")
                for g in range(8):
                    nc.sync.dma_start(out=full[16 * g : 16 * (g + 1), :], in_=pidx16[:, :])
                idx128.append(full)

            xp = pre.tile([128, NSUB], F32)
            yp = pre.tile([128, NSUB], F32)
            nc.sync.dma_start(out=xp[:], in_=bass.AP(coords, 0, [[2, 128], [256, NSUB]]))
            nc.sync.dma_start(out=yp[:], in_=bass.AP(coords, 1, [[2, 128], [256, NSUB]]))
            wts = []
            for si, (C, H, W, inv) in enumerate(SCALES):
                xs = pre.tile([128, NSUB], F32, tag="xs")
                ys = pre.tile([128, NSUB], F32, tag="ys")
                nc.vector.tensor_scalar(xs[:], xp[:], inv, None, OP.mult)
                nc.vector.tensor_scalar(ys[:], yp[:], inv, None, OP.mult)
                _, wx2, wx1 = _floor_pipeline(nc, pre, xs, [128, NSUB], "wx")
                _, wy2, wy1 = _floor_pipeline(nc, pre, ys, [128, NSUB], "wy")
                ws = []
                for (wx, wy, nm) in [
                    (wx1, wy1, "w11"),
                    (wx1, wy2, "w12"),
                    (wx2, wy1, "w21"),
                    (wx2, wy2, "w22"),
                ]:
                    w = pre.tile([128, NSUB], F32, tag=f"{nm}_{si}")
                    nc.vector.tensor_tensor(out=w[:], in0=wx[:], in1=wy[:], op=OP.mult)
                    ws.append(w)
                wts.append(ws)

            pools = [g3p, g4p, g5p]
            NS = OCH // 128
            for c in range(NCHUNK):
                slabs = []
                for si, (C, H, W, inv) in enumerate(SCALES):
                    slab = pools[si].tile([128, NS, 4 * C], BF16, tag=f"slab{si}")
                    i0 = (c * OCH) // 16
                    nc.gpsimd.dma_gather(
                        out_ap=slab[:],
                        in_ap=bass.AP(tabs[si], 0, [[2 * C, (H - 1) * W - 1], [1, 4 * C]]),
                        idxs_ap=idx128[si][:, i0 : i0 + OCH // 16],
                        num_idxs=OCH,
                        num_idxs_reg=OCH,
                        elem_size=4 * C,
                        elem_step=2 * C,
                        queue_num=0,
                    )
                    slabs.append(slab)

                oslab = obp.tile([128, NS, 1280], BF16, tag="oslab")
                for s in range(NS):
                    g = c * NS + s
                    for si, (C, H, W, inv) in enumerate(SCALES):
                        w11, w12, w21, w22 = wts[si]
                        slab = slabs[si]
                        t0 = tmp.tile([128, 512], BF16, tag="t0")
                        t1 = tmp.tile([128, 512], BF16, tag="t1")
                        t2 = tmp.tile([128, 512], BF16, tag="t2")
                        t3 = tmp.tile([128, 512], BF16, tag="t3")
                        nc.vector.tensor_scalar(
                            t0[:, :C], slab[:, s, 0:C], w11[:, g : g + 1], None, OP.mult
                        )
                        nc.scalar.activation(
                            t1[:, :C], slab[:, s, C : 2 * C], AF.Copy, scale=w12[:, g : g + 1]
                        )
                        nc.vector.tensor_scalar(
                            t2[:, :C], slab[:, s, 2 * C : 3 * C], w21[:, g : g + 1], None, OP.mult
                        )
                        nc.scalar.activation(
                            t3[:, :C], slab[:, s, 3 * C : 4 * C], AF.Copy, scale=w22[:, g : g + 1]
                        )
                        nc.vector.tensor_tensor(out=t0[:, :C], in0=t0[:, :C], in1=t1[:, :C], op=OP.add)
                        nc.gpsimd.tensor_tensor(out=t2[:, :C], in0=t2[:, :C], in1=t3[:, :C], op=OP.add)
                        nc.vector.tensor_tensor(
                            out=oslab[:, s, COFF[si] : COFF[si] + C],
                            in0=t0[:, :C],
                            in1=t2[:, :C],
                            op=OP.add,
                        )
                nc.sync.dma_start(
                    out=bass.AP(
                        out,
                        c * OCH * 1280,
                        [[1280, 128], [128 * 1280, NS], [1, 1280]],
                    ),
                    in_=oslab[:],
                )
    nc.compile()
    return nc


def _make_tables(fm):
    C, H, W = fm.shape
    t = np.ascontiguousarray(fm.transpose(1, 2, 0))
    rp = np.concatenate([t[:-1], t[1:]], axis=2)
    return np.ascontiguousarray(rp.reshape((H - 1) * W, 2 * C).astype(ml_dtypes.bfloat16))


def _host_inputs(cb, fm3b, fm4b, fm5b):
    m = {"coords": np.ascontiguousarray(cb)}
    for si, fm in enumerate((fm3b, fm4b, fm5b)):
        m[f"t{si}"] = _make_tables(fm)
    return m


def _host_output(dev_out):
    return np.asarray(dev_out).astype(np.float32)


def kernel(c, fm3, fm4, fm5):
    c = np.asarray(c, np.float32)
    fms = [np.asarray(fm3, np.float32), np.asarray(fm4, np.float32), np.asarray(fm5, np.float32)]
    if "nc" not in _CACHE:
        _CACHE["nc"] = build()
    nc = _CACHE["nc"]
    in_maps = [_host_inputs(c[b], fms[0][b], fms[1][b], fms[2][b]) for b in range(B)]
    res = run_bass_kernel_spmd(nc, in_maps, core_ids=list(range(B)))
    return np.stack([_host_output(res.results[b]["out"]) for b in range(B)], axis=0)
